# revision 46
# baseline (speedup 1.0000x reference)
"""Trainium2 8-core tensor-parallel attention kernel (Bass/Tile).

Sharding: heads tensor-parallel across 8 cores (2 heads/core).
wq/wk/wv column-sharded by head, wo row-sharded; x replicated.
Chunked ReduceScatter (bf16) after the output projection; the host
concatenates the per-core row shards into the full output.

Self-contained: hardcodes B=2, S=2048, DIM=2048, NH=16, HD=128.
"""
import math

import numpy as np

B, S_FULL, DIM, NH = 2, 2048, 2048, 16
HD = 128
N_CORES = 8
HPC = NH // N_CORES          # heads per core
OC = HPC * HD                # output channels per core (256)
DT = DIM // 128              # d-tiles (16)
SC_W = 512                   # schunk width (cols of flattened seq)
RS_ROWS = 512                # rows per ReduceScatter chunk

_CACHE = {}


def _build(S):
    """Build the 8-core SPMD Bass graph for sequence length S (B=2 fixed)."""
    import concourse.bass as bass
    import concourse.mybir as mybir
    import concourse.tile as tile
    from concourse import bacc

    fp32 = mybir.dt.float32
    bf16 = mybir.dt.bfloat16
    Exp = mybir.ActivationFunctionType.Exp
    Copy = mybir.ActivationFunctionType.Copy
    AX = mybir.AxisListType.X
    ADD = mybir.AluOpType.add

    FLAT = B * S                 # flattened rows
    NSC = FLAT // SC_W           # schunks in phase 1
    NQT = S // 128               # q-tiles per batch
    NQG = NQT // 4               # q-groups of 4 tiles per batch
    NCH = FLAT // RS_ROWS        # ReduceScatter chunks
    SCALE = 1.0 / math.sqrt(HD)
    rg = [list(range(N_CORES))]

    nc = bacc.Bacc("TRN2", target_bir_lowering=False, debug=False,
                   num_devices=N_CORES)

    # ---- external parameters ----
    xt_d = nc.declare_dram_parameter("xt", [DIM, FLAT], bf16, isOutput=False)
    wqt_d = nc.declare_dram_parameter("wqt", [DIM, OC], bf16, isOutput=False)
    wkt_d = nc.declare_dram_parameter("wkt", [DIM, OC], bf16, isOutput=False)
    wvt_d = nc.declare_dram_parameter("wvt", [DIM, OC], bf16, isOutput=False)
    wot_d = nc.declare_dram_parameter("wot", [OC, DIM], bf16, isOutput=False)
    cos_d = nc.declare_dram_parameter("cos_t", [HD, S], bf16, isOutput=False)
    sin_d = nc.declare_dram_parameter("sin_t", [HD, S], bf16, isOutput=False)
    mdg_d = nc.declare_dram_parameter("mask_diag", [NQT, 128, 128], fp32, isOutput=False)
    idn_d = nc.declare_dram_parameter("ident_bf", [128, 128], bf16, isOutput=False)
    rot_d = nc.declare_dram_parameter("rotp", [128, 128], bf16, isOutput=False)
    one_d = nc.declare_dram_parameter("ones_bf", [128, 1], bf16, isOutput=False)
    onr_d = nc.declare_dram_parameter("ones_row", [1, 128], fp32, isOutput=False)
    out_d = nc.declare_dram_parameter("out", [NCH, RS_ROWS // N_CORES, DIM], bf16,
                                      isOutput=True)

    # ---- internal DRAM ----
    qT_d = [nc.dram_tensor(f"qT_dram{bb}", [HPC, 128, S], bf16) for bb in range(B)]
    kT_d = [nc.dram_tensor(f"kT_dram{bb}", [HPC, 128, S], bf16) for bb in range(B)]
    vN_d = [nc.dram_tensor(f"vN_dram{bb}", [HPC, S, HD], bf16) for bb in range(B)]
    par_d = [nc.dram_tensor(f"partial_dram{c}", [RS_ROWS, DIM], bf16)
             for c in range(NCH)]
    rs_d = [nc.dram_tensor(f"rs_out{c}", [RS_ROWS // N_CORES, DIM], bf16)
            for c in range(NCH)]

    from contextlib import ExitStack
    with tile.TileContext(nc) as tc:
        with ExitStack() as _stk:
            cpool = _stk.enter_context(tc.tile_pool(name="consts", bufs=1))
            wpool = _stk.enter_context(tc.tile_pool(name="wqkv", bufs=1))
            xpool = _stk.enter_context(tc.tile_pool(name="xT", bufs=18))
            spool = _stk.enter_context(tc.tile_pool(name="p1sb", bufs=3))
            tpool = _stk.enter_context(tc.tile_pool(name="p1tmp", bufs=2))
            qkpool = _stk.enter_context(tc.tile_pool(name="qk_sb", bufs=2))
            vpool = _stk.enter_context(tc.tile_pool(name="vbf", bufs=2))
            ptpool = _stk.enter_context(tc.tile_pool(name="probsT", bufs=2))
            opool = _stk.enter_context(tc.tile_pool(name="outT", bufs=2))
            smpool = _stk.enter_context(tc.tile_pool(name="small", bufs=4))
            papool = _stk.enter_context(tc.tile_pool(name="partial", bufs=3))
            qkvps = tc.alloc_tile_pool(name="qkvps", bufs=6, space="PSUM")
            rotps = tc.alloc_tile_pool(name="rotps", bufs=2, space="PSUM")
            wot_sb = cpool.tile([128, HPC, DIM], bf16)
            nc.gpsimd.dma_start(wot_sb[:], wot_d[:].rearrange("(h p) e -> p h e", p=128))
            cos_sb = cpool.tile([HD, S], bf16)
            nc.gpsimd.dma_start(cos_sb[:], cos_d[:])
            sin_sb = cpool.tile([HD, S], bf16)
            nc.gpsimd.dma_start(sin_sb[:], sin_d[:])
            mdg_sb = cpool.tile([128, NQT, 128], fp32)
            nc.gpsimd.dma_start(mdg_sb[:], mdg_d[:].rearrange("t p k -> p t k"))
            idn_sb = cpool.tile([128, 128], bf16)
            nc.gpsimd.dma_start(idn_sb[:], idn_d[:])
            rot_sb = cpool.tile([128, 128], bf16)
            nc.gpsimd.dma_start(rot_sb[:], rot_d[:])
            one_sb = cpool.tile([128, 1], bf16)
            nc.gpsimd.dma_start(one_sb[:], one_d[:])
            onr_sb = cpool.tile([1, 128], fp32)
            nc.gpsimd.dma_start(onr_sb[:], onr_d[:])

            # ================= phase 1: QKV projections (transposed) ======
            w_sb = {}
            for nm in ("q", "k", "v"):
                w_sb[nm] = wpool.tile([128, DT, OC], bf16, tag=f"w{nm}", name=f"w{nm}")
            # first matmul needs only wq[dt] slabs + x tiles: load those first,
            # one slab at a time, in consumption order
            for dt in range(DT):
                nc.sync.dma_start(w_sb["q"][:, dt, :],
                                  wqt_d[dt * 128:(dt + 1) * 128, :])

            for sc in range(NSC):
                s0 = (sc * SC_W) % S  # position offset within batch
                bb, c0 = divmod(sc * SC_W, S)
                xts = []
                for dt in range(DT):
                    xt = xpool.tile([128, SC_W], bf16, tag="xt", name=f"xt{dt}")
                    nc.sync.dma_start(
                        xt[:], xt_d[dt * 128:(dt + 1) * 128,
                                    sc * SC_W:(sc + 1) * SC_W])
                    xts.append(xt)
                if sc == 0:
                    for dt in range(DT):
                        nc.sync.dma_start(w_sb["k"][:, dt, :],
                                          wkt_d[dt * 128:(dt + 1) * 128, :])
                    for dt in range(DT):
                        nc.sync.dma_start(w_sb["v"][:, dt, :],
                                          wvt_d[dt * 128:(dt + 1) * 128, :])
                for h in range(HPC):
                    ps = {}
                    for t in ("q", "k", "v"):
                        ps[t] = qkvps.tile([128, SC_W], fp32, tag="qkv", name=f"ps_{t}")
                    for t in ("q", "k", "v"):
                        for dt in range(DT):
                            nc.tensor.matmul(
                                ps[t][:],
                                w_sb[t][:, dt, h * HD:(h + 1) * HD],
                                xts[dt][:],
                                start=(dt == 0), stop=(dt == DT - 1))
                    # RoPE for q, k
                    for t, dram, scale in (("q", qT_d, SCALE), ("k", kT_d, 1.0)):
                        til = spool.tile([128, SC_W], bf16, tag="til")
                        nc.scalar.activation(til[:], ps[t][:], Copy,
                                             scale=scale)
                        rp = rotps.tile([128, SC_W], fp32, tag="rot")
                        nc.tensor.matmul(rp[:], rot_sb[:], til[:],
                                         start=True, stop=True)
                        t1 = tpool.tile([128, SC_W], bf16, tag="t1")
                        nc.vector.tensor_mul(t1[:], til[:],
                                             cos_sb[:, s0:s0 + SC_W])
                        hat = spool.tile([128, SC_W], bf16, tag="hat")
                        nc.vector.tensor_mul(hat[:], rp[:],
                                             sin_sb[:, s0:s0 + SC_W])
                        nc.vector.tensor_add(hat[:], hat[:], t1[:])
                        nc.sync.dma_start(dram[bb][h, :, c0:c0 + SC_W], hat[:])
                    # V: copy out of PSUM, then PE-transpose to natural [k, hd]
                    vb = spool.tile([128, SC_W], bf16, tag="vb")
                    nc.scalar.copy(vb[:], ps["v"][:])
                    for vt in range(SC_W // 128):
                        vtp = rotps.tile([128, 128], fp32, tag="rot", name="vtp")
                        nc.tensor.matmul(vtp[:], vb[:, vt * 128:(vt + 1) * 128],
                                         idn_sb[:], start=True, stop=True)
                        vnt = spool.tile([128, 128], bf16, tag="vnt", name="vnt")
                        nc.scalar.copy(vnt[:], vtp[:])
                        nc.sync.dma_start(
                            vN_d[bb][h, c0 + vt * 128: c0 + (vt + 1) * 128, :],
                            vnt[:])

            rotps.release()
            qkvps.release()

            # ================= phase 2: attention + O-proj + RS ===========
            with ExitStack() as _stk2:
                scps = _stk2.enter_context(tc.tile_pool(name="scps", bufs=2, space="PSUM"))
                bps = _stk2.enter_context(tc.tile_pool(name="bps", bufs=1, space="PSUM"))
                sups = _stk2.enter_context(tc.tile_pool(name="sups", bufs=1, space="PSUM"))
                ops = _stk2.enter_context(tc.tile_pool(name="ops", bufs=2, space="PSUM"))
                pps = _stk2.enter_context(tc.tile_pool(name="pps", bufs=2, space="PSUM"))
                def attn_group(qg, kmax, qTh, kTh, vbfh, oTh):
                    pT = ptpool.tile([128, NQT, 512], bf16, tag="pT", name="pT")
                    po = ops.tile([128, 512], fp32, tag="po", name="po")
                    sums_ps = sups.tile([1, 512], fp32, tag="sps", name="sums_ps")
                    for kt in range(kmax + 1):
                        qlo = max(0, kt - qg * 4) * 128
                        n = 512 - qlo
                        sp = scps.tile([128, 512], fp32, tag="sc", name="sp")
                        nc.tensor.matmul(
                            sp[:, :n],
                            kTh[:, kt * 128:(kt + 1) * 128],
                            qTh[:, qg * 512 + qlo: (qg + 1) * 512],
                            start=True, stop=True)
                        if kt >= qg * 4:  # diag tile at local cols 0:128
                            nc.vector.tensor_add(
                                sp[:, 0:128], sp[:, 0:128], mdg_sb[:, kt, :])
                        nc.scalar.activation(
                            pT[:, kt, qlo:512], sp[:, :n], Exp)
                        if kt >= 1:
                            klast = kt - 1
                            ql2 = max(0, klast - qg * 4) * 128
                            nc.tensor.matmul(
                                po[:, ql2:512], vbfh[:, klast, :],
                                pT[:, klast, ql2:512],
                                start=(klast == 0), stop=False)
                            nc.tensor.matmul(
                                sums_ps[:, ql2:512], one_sb[:],
                                pT[:, klast, ql2:512],
                                start=(klast == 0), stop=False)
                    nc.tensor.matmul(
                        po[:, 384:512], vbfh[:, kmax, :],
                        pT[:, kmax, 384:512], start=False, stop=True)
                    nc.tensor.matmul(
                        sums_ps[:, 384:512], one_sb[:],
                        pT[:, kmax, 384:512], start=False, stop=True)
                    # normalization for this q-column group
                    srow = smpool.tile([1, 512], fp32, tag="srow", name="srow")
                    nc.scalar.copy(srow[:], sums_ps[:])
                    sbc_ps = bps.tile([128, 512], fp32, tag="sbc", name="sbc_ps")
                    nc.tensor.matmul(sbc_ps[:], onr_sb[:], srow[:],
                                     start=True, stop=True)
                    rbc = smpool.tile([128, 512], fp32, tag="rbc", name="rbc")
                    nc.vector.reciprocal_approx_fast(rbc[:], sbc_ps[:])
                    nc.vector.tensor_mul(
                        oTh[:, qg * 512:(qg + 1) * 512], po[:], rbc[:])

                qTa, kTa, vbfa = {}, {}, {}
                for bb2 in range(B):
                    for h in range(HPC):
                        qTa[(bb2, h)] = qkpool.tile([128, S], bf16, tag=f"qT{h}", name=f"qT{bb2}{h}")
                        nc.sync.dma_start(qTa[(bb2, h)][:], qT_d[bb2][h])
                        kTa[(bb2, h)] = qkpool.tile([128, S], bf16, tag=f"kT{h}", name=f"kT{bb2}{h}")
                        nc.sync.dma_start(kTa[(bb2, h)][:], kT_d[bb2][h])
                        vbfa[(bb2, h)] = vpool.tile([128, NQT, HD], bf16, tag=f"v{h}", name=f"v{bb2}{h}")
                        nc.sync.dma_start(
                            vbfa[(bb2, h)][:],
                            vN_d[bb2][h].rearrange("(t p) d -> p t d", p=128))

                for b in range(B):
                    qT = {h: qTa[(b, h)] for h in range(HPC)}
                    kT = {h: kTa[(b, h)] for h in range(HPC)}
                    vbf = {h: vbfa[(b, h)] for h in range(HPC)}
                    oT = {}
                    for h in range(HPC):
                        oT[h] = opool.tile([128, S], bf16, tag=f"oT{h}", name=f"oT{h}")

                    qgs = list(range(NQG))
                    for qg in qgs:
                        kmax = qg * 4 + 3
                        for h in range(HPC):
                            attn_group(qg, kmax, qT[h], kT[h], vbf[h], oT[h])

                        # ---- O-projection for this q-group + ReduceScatter ----
                        for st in range(qg * 4, qg * 4 + 4):
                            pp = [pps.tile([128, 512], fp32, tag="pp", name=f"pp{e}") for e in range(4)]
                            for h in range(HPC):
                                for ec in range(4):
                                    nc.tensor.matmul(
                                        pp[ec][:],
                                        oT[h][:, st * 128:(st + 1) * 128],
                                        wot_sb[:, h, ec * 512:(ec + 1) * 512],
                                        start=(h == 0), stop=(h == HPC - 1))
                            par = papool.tile([128, DIM], bf16, tag="par")
                            chx, r0 = divmod(b * S + st * 128, RS_ROWS)
                            for ec in range(4):
                                if ec % 2 == 0:
                                    nc.scalar.copy(par[:, ec * 512:(ec + 1) * 512], pp[ec][:])
                                else:
                                    nc.vector.tensor_copy(par[:, ec * 512:(ec + 1) * 512], pp[ec][:])
                            nc.sync.dma_start(par_d[chx][r0:r0 + 128, :], par[:])
                            if r0 + 128 == RS_ROWS:
                                nc.gpsimd.collective_compute(
                                    "ReduceScatter", ADD, replica_groups=rg,
                                    ins=[par_d[chx][:]],
                                    outs=[rs_d[chx][:]])
                                nc.gpsimd.dma_start(out_d[chx], rs_d[chx][:])


    nc.compile()
    return nc


def _get_nc(S):
    if S not in _CACHE:
        _CACHE[S] = _build(S)
    return _CACHE[S]


def make_inputs(x, freqs_cis, mask, wq, wk, wv, wo):
    """Host-side sharding / layout prep. Returns in_maps for 8 cores."""
    S = x.shape[1]
    flat_xt = np.ascontiguousarray(np.asarray(x, np.float32).reshape(B * S, DIM).T)
    cos = np.asarray(freqs_cis[..., 0], np.float32)   # [S, HD/2]
    sin = np.asarray(freqs_cis[..., 1], np.float32)
    cos_t = np.ascontiguousarray(np.repeat(cos.T, 2, axis=0))  # [HD, S]
    sin_t = np.ascontiguousarray(np.repeat(sin.T, 2, axis=0))
    m = np.asarray(mask, np.float32)[0, 0]
    nqt = S // 128
    mask_diag = np.ascontiguousarray(
        np.stack([m[i * 128:(i + 1) * 128, i * 128:(i + 1) * 128].T
                  for i in range(nqt)]))
    import ml_dtypes
    bf = ml_dtypes.bfloat16
    flat_xt = flat_xt.astype(bf)
    cos_t = cos_t.astype(bf)
    sin_t = sin_t.astype(bf)
    ident_bf = np.eye(128, dtype=bf)
    P = np.zeros((128, 128), np.float32)
    for j in range(64):
        P[2 * j, 2 * j + 1] = -1.0
        P[2 * j + 1, 2 * j] = 1.0
    rotp = np.ascontiguousarray(P.T)

    in_maps = []
    for c in range(N_CORES):
        r = slice(c * OC, (c + 1) * OC)
        in_maps.append({
            "xt": flat_xt,
            "wqt": np.ascontiguousarray(np.asarray(wq, np.float32)[r, :].T).astype(bf),
            "wkt": np.ascontiguousarray(np.asarray(wk, np.float32)[r, :].T).astype(bf),
            "wvt": np.ascontiguousarray(np.asarray(wv, np.float32)[r, :].T).astype(bf),
            "wot": np.ascontiguousarray(np.asarray(wo, np.float32)[:, r].T).astype(bf),
            "cos_t": cos_t,
            "sin_t": sin_t,
            "mask_diag": mask_diag,
            "ident_bf": ident_bf,
            "rotp": rotp.astype(bf),
            "ones_bf": np.ones((128, 1), dtype=bf),
            "ones_row": np.ones((1, 128), dtype=np.float32),
        })
    return in_maps


def assemble(results, S):
    """Concatenate per-core ReduceScatter shards into the full output."""
    nch = B * S // RS_ROWS
    per = RS_ROWS // N_CORES
    full = np.empty((nch, N_CORES, per, DIM), np.float32)
    for c in range(N_CORES):
        full[:, c] = np.asarray(results[c]["out"], np.float32).reshape(nch, per, DIM)
    return full.reshape(B, S, DIM)


def kernel(x, start_pos, freqs_cis, mask, wq, wk, wv, wo):
    from concourse.bass_utils import run_bass_kernel_spmd
    S = x.shape[1]
    nc = _get_nc(S)
    in_maps = make_inputs(x, freqs_cis, mask, wq, wk, wv, wo)
    res = run_bass_kernel_spmd(nc, in_maps, core_ids=list(range(N_CORES)))
    return assemble(res.results, S)


# revision 47
# speedup vs baseline: 1.0111x; 1.0111x over previous
"""Trainium2 8-core tensor-parallel attention kernel (Bass/Tile).

Sharding: heads tensor-parallel across 8 cores (2 heads/core).
wq/wk/wv column-sharded by head, wo row-sharded; x replicated.
Chunked ReduceScatter (bf16) after the output projection; the host
concatenates the per-core row shards into the full output.

Self-contained: hardcodes B=2, S=2048, DIM=2048, NH=16, HD=128.
"""
import math

import numpy as np

B, S_FULL, DIM, NH = 2, 2048, 2048, 16
HD = 128
N_CORES = 8
HPC = NH // N_CORES          # heads per core
OC = HPC * HD                # output channels per core (256)
DT = DIM // 128              # d-tiles (16)
SC_W = 512                   # schunk width (cols of flattened seq)
RS_ROWS = 512                # rows per ReduceScatter chunk

_CACHE = {}


def _build(S):
    """Build the 8-core SPMD Bass graph for sequence length S (B=2 fixed)."""
    import concourse.bass as bass
    import concourse.mybir as mybir
    import concourse.tile as tile
    from concourse import bacc

    fp32 = mybir.dt.float32
    bf16 = mybir.dt.bfloat16
    Exp = mybir.ActivationFunctionType.Exp
    Copy = mybir.ActivationFunctionType.Copy
    AX = mybir.AxisListType.X
    ADD = mybir.AluOpType.add

    FLAT = B * S                 # flattened rows
    NSC = FLAT // SC_W           # schunks in phase 1
    NQT = S // 128               # q-tiles per batch
    NQG = NQT // 4               # q-groups of 4 tiles per batch
    NCH = FLAT // RS_ROWS        # ReduceScatter chunks
    SCALE = 1.0 / math.sqrt(HD)
    rg = [list(range(N_CORES))]

    nc = bacc.Bacc("TRN2", target_bir_lowering=False, debug=False,
                   num_devices=N_CORES)

    # ---- external parameters ----
    xt_d = nc.declare_dram_parameter("xt", [DIM, FLAT], bf16, isOutput=False)
    wqt_d = nc.declare_dram_parameter("wqt", [DIM, OC], bf16, isOutput=False)
    wkt_d = nc.declare_dram_parameter("wkt", [DIM, OC], bf16, isOutput=False)
    wvt_d = nc.declare_dram_parameter("wvt", [DIM, OC], bf16, isOutput=False)
    wot_d = nc.declare_dram_parameter("wot", [OC, DIM], bf16, isOutput=False)
    cos_d = nc.declare_dram_parameter("cos_t", [HD, S], bf16, isOutput=False)
    sin_d = nc.declare_dram_parameter("sin_t", [HD, S], bf16, isOutput=False)
    mdg_d = nc.declare_dram_parameter("mask_diag", [NQT, 128, 128], fp32, isOutput=False)
    idn_d = nc.declare_dram_parameter("ident_bf", [128, 128], bf16, isOutput=False)
    rot_d = nc.declare_dram_parameter("rotp", [128, 128], bf16, isOutput=False)
    one_d = nc.declare_dram_parameter("ones_bf", [128, 1], bf16, isOutput=False)
    onr_d = nc.declare_dram_parameter("ones_row", [1, 128], fp32, isOutput=False)
    out_d = nc.declare_dram_parameter("out", [NCH, RS_ROWS // N_CORES, DIM], bf16,
                                      isOutput=True)

    # ---- internal DRAM ----
    qT_d = [nc.dram_tensor(f"qT_dram{bb}", [HPC, 128, S], bf16) for bb in range(B)]
    kT_d = [nc.dram_tensor(f"kT_dram{bb}", [HPC, 128, S], bf16) for bb in range(B)]
    vN_d = [nc.dram_tensor(f"vN_dram{bb}", [HPC, S, HD], bf16) for bb in range(B)]
    par_d = [nc.dram_tensor(f"partial_dram{c}", [RS_ROWS, DIM], bf16)
             for c in range(NCH)]
    rs_d = [nc.dram_tensor(f"rs_out{c}", [RS_ROWS // N_CORES, DIM], bf16)
            for c in range(NCH)]

    from contextlib import ExitStack
    with tile.TileContext(nc) as tc:
        with ExitStack() as _stk:
            cpool = _stk.enter_context(tc.tile_pool(name="consts", bufs=1))
            wpool = _stk.enter_context(tc.tile_pool(name="wqkv", bufs=1))
            xpool = _stk.enter_context(tc.tile_pool(name="xT", bufs=10))
            spool = _stk.enter_context(tc.tile_pool(name="p1sb", bufs=3))
            tpool = _stk.enter_context(tc.tile_pool(name="p1tmp", bufs=2))
            qkpool = _stk.enter_context(tc.tile_pool(name="qk_sb", bufs=2))
            vpool = _stk.enter_context(tc.tile_pool(name="vbf", bufs=2))
            ptpool = _stk.enter_context(tc.tile_pool(name="probsT", bufs=2))
            opool = _stk.enter_context(tc.tile_pool(name="outT", bufs=2))
            smpool = _stk.enter_context(tc.tile_pool(name="small", bufs=4))
            papool = _stk.enter_context(tc.tile_pool(name="partial", bufs=4))
            qkvps = tc.alloc_tile_pool(name="qkvps", bufs=6, space="PSUM")
            rotps = tc.alloc_tile_pool(name="rotps", bufs=2, space="PSUM")
            wot_sb = cpool.tile([128, HPC, DIM], bf16)
            nc.gpsimd.dma_start(wot_sb[:], wot_d[:].rearrange("(h p) e -> p h e", p=128))
            cos_sb = cpool.tile([HD, S], bf16)
            nc.gpsimd.dma_start(cos_sb[:], cos_d[:])
            sin_sb = cpool.tile([HD, S], bf16)
            nc.gpsimd.dma_start(sin_sb[:], sin_d[:])
            mdg_sb = cpool.tile([128, NQT, 128], fp32)
            nc.gpsimd.dma_start(mdg_sb[:], mdg_d[:].rearrange("t p k -> p t k"))
            idn_sb = cpool.tile([128, 128], bf16)
            nc.gpsimd.dma_start(idn_sb[:], idn_d[:])
            rot_sb = cpool.tile([128, 128], bf16)
            nc.gpsimd.dma_start(rot_sb[:], rot_d[:])
            one_sb = cpool.tile([128, 1], bf16)
            nc.gpsimd.dma_start(one_sb[:], one_d[:])
            onr_sb = cpool.tile([1, 128], fp32)
            nc.gpsimd.dma_start(onr_sb[:], onr_d[:])

            # ================= phase 1: QKV projections (transposed) ======
            w_sb = {}
            for nm in ("q", "k", "v"):
                w_sb[nm] = wpool.tile([128, DT, OC], bf16, tag=f"w{nm}", name=f"w{nm}")
            # first matmul needs only wq[dt] slabs + x tiles: load those first,
            # one slab at a time, in consumption order
            for dt in range(DT):
                nc.sync.dma_start(w_sb["q"][:, dt, :],
                                  wqt_d[dt * 128:(dt + 1) * 128, :])

            for sc in range(NSC):
                s0 = (sc * SC_W) % S  # position offset within batch
                bb, c0 = divmod(sc * SC_W, S)
                xts = []
                for dt in range(DT):
                    xt = xpool.tile([128, SC_W], bf16, tag="xt", name=f"xt{dt}")
                    nc.sync.dma_start(
                        xt[:], xt_d[dt * 128:(dt + 1) * 128,
                                    sc * SC_W:(sc + 1) * SC_W])
                    xts.append(xt)
                if sc == 0:
                    for dt in range(DT):
                        nc.sync.dma_start(w_sb["k"][:, dt, :],
                                          wkt_d[dt * 128:(dt + 1) * 128, :])
                    for dt in range(DT):
                        nc.sync.dma_start(w_sb["v"][:, dt, :],
                                          wvt_d[dt * 128:(dt + 1) * 128, :])
                for h in range(HPC):
                    ps = {}
                    for t in ("q", "k", "v"):
                        ps[t] = qkvps.tile([128, SC_W], fp32, tag="qkv", name=f"ps_{t}")
                    for t in ("q", "k", "v"):
                        for dt in range(DT):
                            nc.tensor.matmul(
                                ps[t][:],
                                w_sb[t][:, dt, h * HD:(h + 1) * HD],
                                xts[dt][:],
                                start=(dt == 0), stop=(dt == DT - 1))
                    # RoPE for q, k
                    for t, dram, scale in (("q", qT_d, SCALE), ("k", kT_d, 1.0)):
                        til = spool.tile([128, SC_W], bf16, tag="til")
                        nc.scalar.activation(til[:], ps[t][:], Copy,
                                             scale=scale)
                        rp = rotps.tile([128, SC_W], fp32, tag="rot")
                        nc.tensor.matmul(rp[:], rot_sb[:], til[:],
                                         start=True, stop=True)
                        t1 = tpool.tile([128, SC_W], bf16, tag="t1")
                        nc.vector.tensor_mul(t1[:], til[:],
                                             cos_sb[:, s0:s0 + SC_W])
                        hat = spool.tile([128, SC_W], bf16, tag="hat")
                        nc.vector.tensor_mul(hat[:], rp[:],
                                             sin_sb[:, s0:s0 + SC_W])
                        nc.vector.tensor_add(hat[:], hat[:], t1[:])
                        nc.sync.dma_start(dram[bb][h, :, c0:c0 + SC_W], hat[:])
                    # V: copy out of PSUM, then PE-transpose to natural [k, hd]
                    vb = spool.tile([128, SC_W], bf16, tag="vb")
                    nc.scalar.copy(vb[:], ps["v"][:])
                    for vt in range(SC_W // 128):
                        vtp = rotps.tile([128, 128], fp32, tag="rot", name="vtp")
                        nc.tensor.matmul(vtp[:], vb[:, vt * 128:(vt + 1) * 128],
                                         idn_sb[:], start=True, stop=True)
                        vnt = spool.tile([128, 128], bf16, tag="vnt", name="vnt")
                        nc.scalar.copy(vnt[:], vtp[:])
                        nc.sync.dma_start(
                            vN_d[bb][h, c0 + vt * 128: c0 + (vt + 1) * 128, :],
                            vnt[:])

            rotps.release()
            qkvps.release()

            # ================= phase 2: attention + O-proj + RS ===========
            with ExitStack() as _stk2:
                scps = _stk2.enter_context(tc.tile_pool(name="scps", bufs=2, space="PSUM"))
                bps = _stk2.enter_context(tc.tile_pool(name="bps", bufs=1, space="PSUM"))
                sups = _stk2.enter_context(tc.tile_pool(name="sups", bufs=1, space="PSUM"))
                ops = _stk2.enter_context(tc.tile_pool(name="ops", bufs=2, space="PSUM"))
                pps = _stk2.enter_context(tc.tile_pool(name="pps", bufs=2, space="PSUM"))
                def attn_group(qg, kmax, qTh, kTh, vbfh, oTh):
                    pT = ptpool.tile([128, NQT, 512], bf16, tag="pT", name="pT")
                    po = ops.tile([128, 512], fp32, tag="po", name="po")
                    sums_ps = sups.tile([1, 512], fp32, tag="sps", name="sums_ps")
                    for kt in range(kmax + 1):
                        qlo = max(0, kt - qg * 4) * 128
                        n = 512 - qlo
                        sp = scps.tile([128, 512], fp32, tag="sc", name="sp")
                        nc.tensor.matmul(
                            sp[:, :n],
                            kTh[:, kt * 128:(kt + 1) * 128],
                            qTh[:, qg * 512 + qlo: (qg + 1) * 512],
                            start=True, stop=True)
                        if kt >= qg * 4:  # diag tile at local cols 0:128
                            nc.vector.tensor_add(
                                sp[:, 0:128], sp[:, 0:128], mdg_sb[:, kt, :])
                        nc.scalar.activation(
                            pT[:, kt, qlo:512], sp[:, :n], Exp)
                        if kt >= 1:
                            klast = kt - 1
                            ql2 = max(0, klast - qg * 4) * 128
                            nc.tensor.matmul(
                                po[:, ql2:512], vbfh[:, klast, :],
                                pT[:, klast, ql2:512],
                                start=(klast == 0), stop=False)
                            nc.tensor.matmul(
                                sums_ps[:, ql2:512], one_sb[:],
                                pT[:, klast, ql2:512],
                                start=(klast == 0), stop=False)
                    nc.tensor.matmul(
                        po[:, 384:512], vbfh[:, kmax, :],
                        pT[:, kmax, 384:512], start=False, stop=True)
                    nc.tensor.matmul(
                        sums_ps[:, 384:512], one_sb[:],
                        pT[:, kmax, 384:512], start=False, stop=True)
                    # normalization for this q-column group
                    srow = smpool.tile([1, 512], fp32, tag="srow", name="srow")
                    nc.scalar.copy(srow[:], sums_ps[:])
                    sbc_ps = bps.tile([128, 512], fp32, tag="sbc", name="sbc_ps")
                    nc.tensor.matmul(sbc_ps[:], onr_sb[:], srow[:],
                                     start=True, stop=True)
                    rbc = smpool.tile([128, 512], fp32, tag="rbc", name="rbc")
                    nc.vector.reciprocal_approx_fast(rbc[:], sbc_ps[:])
                    nc.vector.tensor_mul(
                        oTh[:, qg * 512:(qg + 1) * 512], po[:], rbc[:])

                qTa, kTa, vbfa = {}, {}, {}
                for bb2 in range(B):
                    for h in range(HPC):
                        qTa[(bb2, h)] = qkpool.tile([128, S], bf16, tag=f"qT{h}", name=f"qT{bb2}{h}")
                        nc.sync.dma_start(qTa[(bb2, h)][:], qT_d[bb2][h])
                        kTa[(bb2, h)] = qkpool.tile([128, S], bf16, tag=f"kT{h}", name=f"kT{bb2}{h}")
                        nc.sync.dma_start(kTa[(bb2, h)][:], kT_d[bb2][h])
                        vbfa[(bb2, h)] = vpool.tile([128, NQT, HD], bf16, tag=f"v{h}", name=f"v{bb2}{h}")
                        nc.sync.dma_start(
                            vbfa[(bb2, h)][:],
                            vN_d[bb2][h].rearrange("(t p) d -> p t d", p=128))

                for b in range(B):
                    qT = {h: qTa[(b, h)] for h in range(HPC)}
                    kT = {h: kTa[(b, h)] for h in range(HPC)}
                    vbf = {h: vbfa[(b, h)] for h in range(HPC)}
                    oT = {}
                    for h in range(HPC):
                        oT[h] = opool.tile([128, S], bf16, tag=f"oT{h}", name=f"oT{h}")

                    qgs = list(range(NQG))
                    for qg in qgs:
                        kmax = qg * 4 + 3
                        for h in range(HPC):
                            attn_group(qg, kmax, qT[h], kT[h], vbf[h], oT[h])

                        # ---- O-projection for this q-group + ReduceScatter ----
                        for st in range(qg * 4, qg * 4 + 4):
                            pp = [pps.tile([128, 512], fp32, tag="pp", name=f"pp{e}") for e in range(4)]
                            for h in range(HPC):
                                for ec in range(4):
                                    nc.tensor.matmul(
                                        pp[ec][:],
                                        oT[h][:, st * 128:(st + 1) * 128],
                                        wot_sb[:, h, ec * 512:(ec + 1) * 512],
                                        start=(h == 0), stop=(h == HPC - 1))
                            par = papool.tile([128, DIM], bf16, tag="par")
                            chx, r0 = divmod(b * S + st * 128, RS_ROWS)
                            for ec in range(4):
                                if ec % 2 == 0:
                                    nc.scalar.copy(par[:, ec * 512:(ec + 1) * 512], pp[ec][:])
                                else:
                                    nc.vector.tensor_copy(par[:, ec * 512:(ec + 1) * 512], pp[ec][:])
                            nc.sync.dma_start(par_d[chx][r0:r0 + 128, :], par[:])
                            if r0 + 128 == RS_ROWS:
                                nc.gpsimd.collective_compute(
                                    "ReduceScatter", ADD, replica_groups=rg,
                                    ins=[par_d[chx][:]],
                                    outs=[rs_d[chx][:]])
                                nc.gpsimd.dma_start(out_d[chx], rs_d[chx][:])


    nc.compile()
    return nc


def _get_nc(S):
    if S not in _CACHE:
        _CACHE[S] = _build(S)
    return _CACHE[S]


def make_inputs(x, freqs_cis, mask, wq, wk, wv, wo):
    """Host-side sharding / layout prep. Returns in_maps for 8 cores."""
    S = x.shape[1]
    flat_xt = np.ascontiguousarray(np.asarray(x, np.float32).reshape(B * S, DIM).T)
    cos = np.asarray(freqs_cis[..., 0], np.float32)   # [S, HD/2]
    sin = np.asarray(freqs_cis[..., 1], np.float32)
    cos_t = np.ascontiguousarray(np.repeat(cos.T, 2, axis=0))  # [HD, S]
    sin_t = np.ascontiguousarray(np.repeat(sin.T, 2, axis=0))
    m = np.asarray(mask, np.float32)[0, 0]
    nqt = S // 128
    mask_diag = np.ascontiguousarray(
        np.stack([m[i * 128:(i + 1) * 128, i * 128:(i + 1) * 128].T
                  for i in range(nqt)]))
    import ml_dtypes
    bf = ml_dtypes.bfloat16
    flat_xt = flat_xt.astype(bf)
    cos_t = cos_t.astype(bf)
    sin_t = sin_t.astype(bf)
    ident_bf = np.eye(128, dtype=bf)
    P = np.zeros((128, 128), np.float32)
    for j in range(64):
        P[2 * j, 2 * j + 1] = -1.0
        P[2 * j + 1, 2 * j] = 1.0
    rotp = np.ascontiguousarray(P.T)

    in_maps = []
    for c in range(N_CORES):
        r = slice(c * OC, (c + 1) * OC)
        in_maps.append({
            "xt": flat_xt,
            "wqt": np.ascontiguousarray(np.asarray(wq, np.float32)[r, :].T).astype(bf),
            "wkt": np.ascontiguousarray(np.asarray(wk, np.float32)[r, :].T).astype(bf),
            "wvt": np.ascontiguousarray(np.asarray(wv, np.float32)[r, :].T).astype(bf),
            "wot": np.ascontiguousarray(np.asarray(wo, np.float32)[:, r].T).astype(bf),
            "cos_t": cos_t,
            "sin_t": sin_t,
            "mask_diag": mask_diag,
            "ident_bf": ident_bf,
            "rotp": rotp.astype(bf),
            "ones_bf": np.ones((128, 1), dtype=bf),
            "ones_row": np.ones((1, 128), dtype=np.float32),
        })
    return in_maps


def assemble(results, S):
    """Concatenate per-core ReduceScatter shards into the full output."""
    nch = B * S // RS_ROWS
    per = RS_ROWS // N_CORES
    full = np.empty((nch, N_CORES, per, DIM), np.float32)
    for c in range(N_CORES):
        full[:, c] = np.asarray(results[c]["out"], np.float32).reshape(nch, per, DIM)
    return full.reshape(B, S, DIM)


def kernel(x, start_pos, freqs_cis, mask, wq, wk, wv, wo):
    from concourse.bass_utils import run_bass_kernel_spmd
    S = x.shape[1]
    nc = _get_nc(S)
    in_maps = make_inputs(x, freqs_cis, mask, wq, wk, wv, wo)
    res = run_bass_kernel_spmd(nc, in_maps, core_ids=list(range(N_CORES)))
    return assemble(res.results, S)


# revision 48
# speedup vs baseline: 1.0159x; 1.0048x over previous
"""Trainium2 8-core tensor-parallel attention kernel (Bass/Tile).

Sharding: heads tensor-parallel across 8 cores (2 heads/core).
wq/wk/wv column-sharded by head, wo row-sharded; x replicated.
Chunked ReduceScatter (bf16) after the output projection; the host
concatenates the per-core row shards into the full output.

Self-contained: hardcodes B=2, S=2048, DIM=2048, NH=16, HD=128.
"""
import math

import numpy as np

B, S_FULL, DIM, NH = 2, 2048, 2048, 16
HD = 128
N_CORES = 8
HPC = NH // N_CORES          # heads per core
OC = HPC * HD                # output channels per core (256)
DT = DIM // 128              # d-tiles (16)
SC_W = 512                   # schunk width (cols of flattened seq)
RS_ROWS = 512                # rows per ReduceScatter chunk

_CACHE = {}


def _build(S):
    """Build the 8-core SPMD Bass graph for sequence length S (B=2 fixed)."""
    import concourse.bass as bass
    import concourse.mybir as mybir
    import concourse.tile as tile
    from concourse import bacc

    fp32 = mybir.dt.float32
    bf16 = mybir.dt.bfloat16
    Exp = mybir.ActivationFunctionType.Exp
    Copy = mybir.ActivationFunctionType.Copy
    AX = mybir.AxisListType.X
    ADD = mybir.AluOpType.add

    FLAT = B * S                 # flattened rows
    NSC = FLAT // SC_W           # schunks in phase 1
    NQT = S // 128               # q-tiles per batch
    NQG = NQT // 4               # q-groups of 4 tiles per batch
    NCH = FLAT // RS_ROWS        # ReduceScatter chunks
    SCALE = 1.0 / math.sqrt(HD)
    rg = [list(range(N_CORES))]

    nc = bacc.Bacc("TRN2", target_bir_lowering=False, debug=False,
                   num_devices=N_CORES)

    # ---- external parameters ----
    xt_d = nc.declare_dram_parameter("xt", [DIM, FLAT], bf16, isOutput=False)
    wqt_d = nc.declare_dram_parameter("wqt", [DIM, OC], bf16, isOutput=False)
    wkt_d = nc.declare_dram_parameter("wkt", [DIM, OC], bf16, isOutput=False)
    wvt_d = nc.declare_dram_parameter("wvt", [DIM, OC], bf16, isOutput=False)
    wot_d = nc.declare_dram_parameter("wot", [OC, DIM], bf16, isOutput=False)
    cos_d = nc.declare_dram_parameter("cos_t", [HD, S], bf16, isOutput=False)
    sin_d = nc.declare_dram_parameter("sin_t", [HD, S], bf16, isOutput=False)
    mdg_d = nc.declare_dram_parameter("mask_diag", [NQT, 128, 128], fp32, isOutput=False)
    idn_d = nc.declare_dram_parameter("ident_bf", [128, 128], bf16, isOutput=False)
    rot_d = nc.declare_dram_parameter("rotp", [128, 128], bf16, isOutput=False)
    one_d = nc.declare_dram_parameter("ones_bf", [128, 1], bf16, isOutput=False)
    onr_d = nc.declare_dram_parameter("ones_row", [1, 128], fp32, isOutput=False)
    out_d = nc.declare_dram_parameter("out", [NCH, RS_ROWS // N_CORES, DIM], bf16,
                                      isOutput=True)

    # ---- internal DRAM ----
    qT_d = [nc.dram_tensor(f"qT_dram{bb}", [HPC, 128, S], bf16) for bb in range(B)]
    kT_d = [nc.dram_tensor(f"kT_dram{bb}", [HPC, 128, S], bf16) for bb in range(B)]
    vN_d = [nc.dram_tensor(f"vN_dram{bb}", [HPC, S, HD], bf16) for bb in range(B)]
    par_d = [nc.dram_tensor(f"partial_dram{c}", [RS_ROWS, DIM], bf16)
             for c in range(NCH)]
    rs_d = [nc.dram_tensor(f"rs_out{c}", [RS_ROWS // N_CORES, DIM], bf16)
            for c in range(NCH)]

    from contextlib import ExitStack
    with tile.TileContext(nc) as tc:
        with ExitStack() as _stk:
            cpool = _stk.enter_context(tc.tile_pool(name="consts", bufs=1))
            wpool = _stk.enter_context(tc.tile_pool(name="wqkv", bufs=1))
            xpool = _stk.enter_context(tc.tile_pool(name="xT", bufs=10))
            spool = _stk.enter_context(tc.tile_pool(name="p1sb", bufs=3))
            tpool = _stk.enter_context(tc.tile_pool(name="p1tmp", bufs=2))
            qkpool = _stk.enter_context(tc.tile_pool(name="qk_sb", bufs=2))
            vpool = _stk.enter_context(tc.tile_pool(name="vbf", bufs=2))
            ptpool = _stk.enter_context(tc.tile_pool(name="probsT", bufs=2))
            opool = _stk.enter_context(tc.tile_pool(name="outT", bufs=2))
            smpool = _stk.enter_context(tc.tile_pool(name="small", bufs=4))
            papool = _stk.enter_context(tc.tile_pool(name="partial", bufs=4))
            qkvps = tc.alloc_tile_pool(name="qkvps", bufs=6, space="PSUM")
            rotps = tc.alloc_tile_pool(name="rotps", bufs=2, space="PSUM")
            wot_sb = cpool.tile([128, HPC, DIM], bf16)
            nc.gpsimd.dma_start(wot_sb[:], wot_d[:].rearrange("(h p) e -> p h e", p=128))
            cos_sb = cpool.tile([HD, S], bf16)
            nc.gpsimd.dma_start(cos_sb[:], cos_d[:])
            sin_sb = cpool.tile([HD, S], bf16)
            nc.gpsimd.dma_start(sin_sb[:], sin_d[:])
            mdg_sb = cpool.tile([128, NQT, 128], fp32)
            nc.gpsimd.dma_start(mdg_sb[:], mdg_d[:].rearrange("t p k -> p t k"))
            idn_sb = cpool.tile([128, 128], bf16)
            nc.gpsimd.dma_start(idn_sb[:], idn_d[:])
            rot_sb = cpool.tile([128, 128], bf16)
            nc.gpsimd.dma_start(rot_sb[:], rot_d[:])
            one_sb = cpool.tile([128, 1], bf16)
            nc.gpsimd.dma_start(one_sb[:], one_d[:])
            onr_sb = cpool.tile([1, 128], fp32)
            nc.gpsimd.dma_start(onr_sb[:], onr_d[:])

            # ================= phase 1: QKV projections (transposed) ======
            w_sb = {}
            for nm in ("q", "k", "v"):
                w_sb[nm] = wpool.tile([128, DT, OC], bf16, tag=f"w{nm}", name=f"w{nm}")
            # first matmul needs only wq[dt] slabs + x tiles: load those first,
            # one slab at a time, in consumption order
            for dt in range(DT):
                nc.sync.dma_start(w_sb["q"][:, dt, :],
                                  wqt_d[dt * 128:(dt + 1) * 128, :])

            for sc in range(NSC):
                s0 = (sc * SC_W) % S  # position offset within batch
                bb, c0 = divmod(sc * SC_W, S)
                xts = []
                for dt in range(DT):
                    xt = xpool.tile([128, SC_W], bf16, tag="xt", name=f"xt{dt}")
                    nc.sync.dma_start(
                        xt[:], xt_d[dt * 128:(dt + 1) * 128,
                                    sc * SC_W:(sc + 1) * SC_W])
                    xts.append(xt)
                if sc == 0:
                    for dt in range(DT):
                        nc.sync.dma_start(w_sb["k"][:, dt, :],
                                          wkt_d[dt * 128:(dt + 1) * 128, :])
                    for dt in range(DT):
                        nc.sync.dma_start(w_sb["v"][:, dt, :],
                                          wvt_d[dt * 128:(dt + 1) * 128, :])
                for h in range(HPC):
                    ps = {}
                    for t in ("q", "k", "v"):
                        ps[t] = qkvps.tile([128, SC_W], fp32, tag="qkv", name=f"ps_{t}")
                    for t in ("q", "k", "v"):
                        for dt in range(DT):
                            nc.tensor.matmul(
                                ps[t][:],
                                w_sb[t][:, dt, h * HD:(h + 1) * HD],
                                xts[dt][:],
                                start=(dt == 0), stop=(dt == DT - 1))
                    # RoPE for q, k
                    for t, dram, scale in (("q", qT_d, SCALE), ("k", kT_d, 1.0)):
                        til = spool.tile([128, SC_W], bf16, tag="til")
                        nc.scalar.activation(til[:], ps[t][:], Copy,
                                             scale=scale)
                        rp = rotps.tile([128, SC_W], fp32, tag="rot")
                        nc.tensor.matmul(rp[:], rot_sb[:], til[:],
                                         start=True, stop=True)
                        t1 = tpool.tile([128, SC_W], bf16, tag="t1")
                        nc.vector.tensor_mul(t1[:], til[:],
                                             cos_sb[:, s0:s0 + SC_W])
                        hat = spool.tile([128, SC_W], bf16, tag="hat")
                        nc.vector.tensor_mul(hat[:], rp[:],
                                             sin_sb[:, s0:s0 + SC_W])
                        nc.vector.tensor_add(hat[:], hat[:], t1[:])
                        nc.sync.dma_start(dram[bb][h, :, c0:c0 + SC_W], hat[:])
                    # V: copy out of PSUM, then PE-transpose to natural [k, hd]
                    vb = spool.tile([128, SC_W], bf16, tag="vb")
                    nc.scalar.copy(vb[:], ps["v"][:])
                    for vt in range(SC_W // 128):
                        vtp = rotps.tile([128, 128], fp32, tag="rot", name="vtp")
                        nc.tensor.matmul(vtp[:], vb[:, vt * 128:(vt + 1) * 128],
                                         idn_sb[:], start=True, stop=True)
                        vnt = spool.tile([128, 128], bf16, tag="vnt", name="vnt")
                        nc.scalar.copy(vnt[:], vtp[:])
                        nc.sync.dma_start(
                            vN_d[bb][h, c0 + vt * 128: c0 + (vt + 1) * 128, :],
                            vnt[:])

            rotps.release()
            qkvps.release()

            # ================= phase 2: attention + O-proj + RS ===========
            with ExitStack() as _stk2:
                scps = _stk2.enter_context(tc.tile_pool(name="scps", bufs=2, space="PSUM"))
                bps = _stk2.enter_context(tc.tile_pool(name="bps", bufs=1, space="PSUM"))
                sups = _stk2.enter_context(tc.tile_pool(name="sups", bufs=1, space="PSUM"))
                ops = _stk2.enter_context(tc.tile_pool(name="ops", bufs=2, space="PSUM"))
                pps = _stk2.enter_context(tc.tile_pool(name="pps", bufs=2, space="PSUM"))
                def attn_group2(qg, kmax, qT, kT, vbf, oT):
                    pT = {h: ptpool.tile([128, NQT, 512], bf16, tag="pT",
                                         name=f"pT{h}") for h in range(HPC)}
                    po = {h: ops.tile([128, 512], fp32, tag="po",
                                      name=f"po{h}") for h in range(HPC)}
                    for kt in range(kmax + 1):
                        qlo = max(0, kt - qg * 4) * 128
                        n = 512 - qlo
                        for h in range(HPC):
                            sp = scps.tile([128, 512], fp32, tag="sc", name="sp")
                            nc.tensor.matmul(
                                sp[:, :n],
                                kT[h][:, kt * 128:(kt + 1) * 128],
                                qT[h][:, qg * 512 + qlo: (qg + 1) * 512],
                                start=True, stop=True)
                            if kt >= qg * 4:  # diag tile at local cols 0:128
                                nc.vector.tensor_add(
                                    sp[:, 0:128], sp[:, 0:128], mdg_sb[:, kt, :])
                            nc.scalar.activation(
                                pT[h][:, kt, qlo:512], sp[:, :n], Exp)
                            if kt >= 1:
                                klast = kt - 1
                                ql2 = max(0, klast - qg * 4) * 128
                                nc.tensor.matmul(
                                    po[h][:, ql2:512], vbf[h][:, klast, :],
                                    pT[h][:, klast, ql2:512],
                                    start=(klast == 0), stop=False)
                    for h in range(HPC):
                        nc.tensor.matmul(
                            po[h][:, 384:512], vbf[h][:, kmax, :],
                            pT[h][:, kmax, 384:512], start=False, stop=True)
                        sums_ps = sups.tile([1, 512], fp32, tag="sps", name="sums_ps")
                        for kt in range(kmax + 1):
                            qlo = max(0, kt - qg * 4) * 128
                            nc.tensor.matmul(
                                sums_ps[:, qlo:512], one_sb[:],
                                pT[h][:, kt, qlo:512],
                                start=(kt == 0), stop=(kt == kmax))
                        srow = smpool.tile([1, 512], fp32, tag="srow", name="srow")
                        nc.scalar.copy(srow[:], sums_ps[:])
                        sbc_ps = bps.tile([128, 512], fp32, tag="sbc", name="sbc_ps")
                        nc.tensor.matmul(sbc_ps[:], onr_sb[:], srow[:],
                                         start=True, stop=True)
                        rbc = smpool.tile([128, 512], fp32, tag="rbc", name="rbc")
                        nc.vector.reciprocal_approx_fast(rbc[:], sbc_ps[:])
                        nc.vector.tensor_mul(
                            oT[h][:, qg * 512:(qg + 1) * 512], po[h][:], rbc[:])

                qTa, kTa, vbfa = {}, {}, {}
                for bb2 in range(B):
                    for h in range(HPC):
                        qTa[(bb2, h)] = qkpool.tile([128, S], bf16, tag=f"qT{h}", name=f"qT{bb2}{h}")
                        nc.sync.dma_start(qTa[(bb2, h)][:], qT_d[bb2][h])
                        kTa[(bb2, h)] = qkpool.tile([128, S], bf16, tag=f"kT{h}", name=f"kT{bb2}{h}")
                        nc.sync.dma_start(kTa[(bb2, h)][:], kT_d[bb2][h])
                        vbfa[(bb2, h)] = vpool.tile([128, NQT, HD], bf16, tag=f"v{h}", name=f"v{bb2}{h}")
                        nc.sync.dma_start(
                            vbfa[(bb2, h)][:],
                            vN_d[bb2][h].rearrange("(t p) d -> p t d", p=128))

                for b in range(B):
                    qT = {h: qTa[(b, h)] for h in range(HPC)}
                    kT = {h: kTa[(b, h)] for h in range(HPC)}
                    vbf = {h: vbfa[(b, h)] for h in range(HPC)}
                    oT = {}
                    for h in range(HPC):
                        oT[h] = opool.tile([128, S], bf16, tag=f"oT{h}", name=f"oT{h}")

                    qgs = list(range(NQG))
                    for qg in qgs:
                        kmax = qg * 4 + 3
                        attn_group2(qg, kmax, qT, kT, vbf, oT)

                        # ---- O-projection for this q-group + ReduceScatter ----
                        for st in range(qg * 4, qg * 4 + 4):
                            pp = [pps.tile([128, 512], fp32, tag="pp", name=f"pp{e}") for e in range(4)]
                            for h in range(HPC):
                                for ec in range(4):
                                    nc.tensor.matmul(
                                        pp[ec][:],
                                        oT[h][:, st * 128:(st + 1) * 128],
                                        wot_sb[:, h, ec * 512:(ec + 1) * 512],
                                        start=(h == 0), stop=(h == HPC - 1))
                            par = papool.tile([128, DIM], bf16, tag="par")
                            chx, r0 = divmod(b * S + st * 128, RS_ROWS)
                            for ec in range(4):
                                if ec % 2 == 0:
                                    nc.scalar.copy(par[:, ec * 512:(ec + 1) * 512], pp[ec][:])
                                else:
                                    nc.vector.tensor_copy(par[:, ec * 512:(ec + 1) * 512], pp[ec][:])
                            nc.sync.dma_start(par_d[chx][r0:r0 + 128, :], par[:])
                            if r0 + 128 == RS_ROWS:
                                nc.gpsimd.collective_compute(
                                    "ReduceScatter", ADD, replica_groups=rg,
                                    ins=[par_d[chx][:]],
                                    outs=[rs_d[chx][:]])
                                nc.gpsimd.dma_start(out_d[chx], rs_d[chx][:])


    nc.compile()
    return nc


def _get_nc(S):
    if S not in _CACHE:
        _CACHE[S] = _build(S)
    return _CACHE[S]


def make_inputs(x, freqs_cis, mask, wq, wk, wv, wo):
    """Host-side sharding / layout prep. Returns in_maps for 8 cores."""
    S = x.shape[1]
    flat_xt = np.ascontiguousarray(np.asarray(x, np.float32).reshape(B * S, DIM).T)
    cos = np.asarray(freqs_cis[..., 0], np.float32)   # [S, HD/2]
    sin = np.asarray(freqs_cis[..., 1], np.float32)
    cos_t = np.ascontiguousarray(np.repeat(cos.T, 2, axis=0))  # [HD, S]
    sin_t = np.ascontiguousarray(np.repeat(sin.T, 2, axis=0))
    m = np.asarray(mask, np.float32)[0, 0]
    nqt = S // 128
    mask_diag = np.ascontiguousarray(
        np.stack([m[i * 128:(i + 1) * 128, i * 128:(i + 1) * 128].T
                  for i in range(nqt)]))
    import ml_dtypes
    bf = ml_dtypes.bfloat16
    flat_xt = flat_xt.astype(bf)
    cos_t = cos_t.astype(bf)
    sin_t = sin_t.astype(bf)
    ident_bf = np.eye(128, dtype=bf)
    P = np.zeros((128, 128), np.float32)
    for j in range(64):
        P[2 * j, 2 * j + 1] = -1.0
        P[2 * j + 1, 2 * j] = 1.0
    rotp = np.ascontiguousarray(P.T)

    in_maps = []
    for c in range(N_CORES):
        r = slice(c * OC, (c + 1) * OC)
        in_maps.append({
            "xt": flat_xt,
            "wqt": np.ascontiguousarray(np.asarray(wq, np.float32)[r, :].T).astype(bf),
            "wkt": np.ascontiguousarray(np.asarray(wk, np.float32)[r, :].T).astype(bf),
            "wvt": np.ascontiguousarray(np.asarray(wv, np.float32)[r, :].T).astype(bf),
            "wot": np.ascontiguousarray(np.asarray(wo, np.float32)[:, r].T).astype(bf),
            "cos_t": cos_t,
            "sin_t": sin_t,
            "mask_diag": mask_diag,
            "ident_bf": ident_bf,
            "rotp": rotp.astype(bf),
            "ones_bf": np.ones((128, 1), dtype=bf),
            "ones_row": np.ones((1, 128), dtype=np.float32),
        })
    return in_maps


def assemble(results, S):
    """Concatenate per-core ReduceScatter shards into the full output."""
    nch = B * S // RS_ROWS
    per = RS_ROWS // N_CORES
    full = np.empty((nch, N_CORES, per, DIM), np.float32)
    for c in range(N_CORES):
        full[:, c] = np.asarray(results[c]["out"], np.float32).reshape(nch, per, DIM)
    return full.reshape(B, S, DIM)


def kernel(x, start_pos, freqs_cis, mask, wq, wk, wv, wo):
    from concourse.bass_utils import run_bass_kernel_spmd
    S = x.shape[1]
    nc = _get_nc(S)
    in_maps = make_inputs(x, freqs_cis, mask, wq, wk, wv, wo)
    res = run_bass_kernel_spmd(nc, in_maps, core_ids=list(range(N_CORES)))
    return assemble(res.results, S)


# revision 49
# speedup vs baseline: 1.0246x; 1.0085x over previous
"""Trainium2 8-core tensor-parallel attention kernel (Bass/Tile).

Sharding: heads tensor-parallel across 8 cores (2 heads/core).
wq/wk/wv column-sharded by head, wo row-sharded; x replicated.
Chunked ReduceScatter (bf16) after the output projection; the host
concatenates the per-core row shards into the full output.

Self-contained: hardcodes B=2, S=2048, DIM=2048, NH=16, HD=128.
"""
import math

import numpy as np

B, S_FULL, DIM, NH = 2, 2048, 2048, 16
HD = 128
N_CORES = 8
HPC = NH // N_CORES          # heads per core
OC = HPC * HD                # output channels per core (256)
DT = DIM // 128              # d-tiles (16)
SC_W = 512                   # schunk width (cols of flattened seq)
RS_ROWS = 512                # rows per ReduceScatter chunk

_CACHE = {}


def _build(S):
    """Build the 8-core SPMD Bass graph for sequence length S (B=2 fixed)."""
    import concourse.bass as bass
    import concourse.mybir as mybir
    import concourse.tile as tile
    from concourse import bacc

    fp32 = mybir.dt.float32
    bf16 = mybir.dt.bfloat16
    Exp = mybir.ActivationFunctionType.Exp
    Copy = mybir.ActivationFunctionType.Copy
    AX = mybir.AxisListType.X
    ADD = mybir.AluOpType.add

    FLAT = B * S                 # flattened rows
    NSC = FLAT // SC_W           # schunks in phase 1
    NQT = S // 128               # q-tiles per batch
    NQG = NQT // 4               # q-groups of 4 tiles per batch
    NCH = FLAT // RS_ROWS        # ReduceScatter chunks
    SCALE = 1.0 / math.sqrt(HD)
    rg = [list(range(N_CORES))]

    nc = bacc.Bacc("TRN2", target_bir_lowering=False, debug=False,
                   num_devices=N_CORES)

    # ---- external parameters ----
    xt_d = nc.declare_dram_parameter("xt", [DIM, FLAT], bf16, isOutput=False)
    wqt_d = nc.declare_dram_parameter("wqt", [DIM, OC], bf16, isOutput=False)
    wkt_d = nc.declare_dram_parameter("wkt", [DIM, OC], bf16, isOutput=False)
    wvt_d = nc.declare_dram_parameter("wvt", [DIM, OC], bf16, isOutput=False)
    wot_d = nc.declare_dram_parameter("wot", [OC, DIM], bf16, isOutput=False)
    cos_d = nc.declare_dram_parameter("cos_t", [HD, S], bf16, isOutput=False)
    sin_d = nc.declare_dram_parameter("sin_t", [HD, S], bf16, isOutput=False)
    mdg_d = nc.declare_dram_parameter("mask_diag", [NQT, 128, 128], fp32, isOutput=False)
    idn_d = nc.declare_dram_parameter("ident_bf", [128, 128], bf16, isOutput=False)
    rot_d = nc.declare_dram_parameter("rotp", [128, 128], bf16, isOutput=False)
    one_d = nc.declare_dram_parameter("ones_bf", [128, 1], bf16, isOutput=False)
    onr_d = nc.declare_dram_parameter("ones_row", [1, 128], fp32, isOutput=False)
    out_d = nc.declare_dram_parameter("out", [NCH, RS_ROWS // N_CORES, DIM], bf16,
                                      isOutput=True)

    # ---- internal DRAM ----
    qT_d = [nc.dram_tensor(f"qT_dram{bb}", [HPC, 128, S], bf16) for bb in range(B)]
    kT_d = [nc.dram_tensor(f"kT_dram{bb}", [HPC, 128, S], bf16) for bb in range(B)]
    vN_d = [nc.dram_tensor(f"vN_dram{bb}", [HPC, S, HD], bf16) for bb in range(B)]
    par_d = [nc.dram_tensor(f"partial_dram{c}", [RS_ROWS, DIM], bf16)
             for c in range(NCH)]
    rs_d = [nc.dram_tensor(f"rs_out{c}", [RS_ROWS // N_CORES, DIM], bf16)
            for c in range(NCH)]

    from contextlib import ExitStack
    with tile.TileContext(nc) as tc:
        with ExitStack() as _stk:
            cpool = _stk.enter_context(tc.tile_pool(name="consts", bufs=1))
            wpool = _stk.enter_context(tc.tile_pool(name="wqkv", bufs=1))
            xpool = _stk.enter_context(tc.tile_pool(name="xT", bufs=10))
            spool = _stk.enter_context(tc.tile_pool(name="p1sb", bufs=3))
            tpool = _stk.enter_context(tc.tile_pool(name="p1tmp", bufs=2))
            qkpool = _stk.enter_context(tc.tile_pool(name="qk_sb", bufs=2))
            vpool = _stk.enter_context(tc.tile_pool(name="vbf", bufs=2))
            ptpool = _stk.enter_context(tc.tile_pool(name="probsT", bufs=2))
            opool = _stk.enter_context(tc.tile_pool(name="outT", bufs=2))
            smpool = _stk.enter_context(tc.tile_pool(name="small", bufs=4))
            papool = _stk.enter_context(tc.tile_pool(name="partial", bufs=4))
            qkvps = tc.alloc_tile_pool(name="qkvps", bufs=6, space="PSUM")
            rotps = tc.alloc_tile_pool(name="rotps", bufs=2, space="PSUM")
            wot_sb = cpool.tile([128, HPC, DIM], bf16)
            nc.gpsimd.dma_start(wot_sb[:], wot_d[:].rearrange("(h p) e -> p h e", p=128))
            cos_sb = cpool.tile([HD, S], bf16)
            nc.gpsimd.dma_start(cos_sb[:], cos_d[:])
            sin_sb = cpool.tile([HD, S], bf16)
            nc.gpsimd.dma_start(sin_sb[:], sin_d[:])
            mdg_sb = cpool.tile([128, NQT, 128], fp32)
            nc.gpsimd.dma_start(mdg_sb[:], mdg_d[:].rearrange("t p k -> p t k"))
            idn_sb = cpool.tile([128, 128], bf16)
            nc.gpsimd.dma_start(idn_sb[:], idn_d[:])
            rot_sb = cpool.tile([128, 128], bf16)
            nc.gpsimd.dma_start(rot_sb[:], rot_d[:])
            one_sb = cpool.tile([128, 1], bf16)
            nc.gpsimd.dma_start(one_sb[:], one_d[:])
            onr_sb = cpool.tile([1, 128], fp32)
            nc.gpsimd.dma_start(onr_sb[:], onr_d[:])

            # ================= phase 1: QKV projections (transposed) ======
            w_sb = {}
            for nm in ("q", "k", "v"):
                w_sb[nm] = wpool.tile([128, DT, OC], bf16, tag=f"w{nm}", name=f"w{nm}")
            # first matmul needs only wq[dt] slabs + x tiles: load those first,
            # one slab at a time, in consumption order
            for dt in range(DT):
                nc.sync.dma_start(w_sb["q"][:, dt, :],
                                  wqt_d[dt * 128:(dt + 1) * 128, :])

            for sc in range(NSC):
                s0 = (sc * SC_W) % S  # position offset within batch
                bb, c0 = divmod(sc * SC_W, S)
                xts = []
                for dt in range(DT):
                    xt = xpool.tile([128, SC_W], bf16, tag="xt", name=f"xt{dt}")
                    nc.sync.dma_start(
                        xt[:], xt_d[dt * 128:(dt + 1) * 128,
                                    sc * SC_W:(sc + 1) * SC_W])
                    xts.append(xt)
                if sc == 0:
                    for dt in range(DT):
                        nc.sync.dma_start(w_sb["k"][:, dt, :],
                                          wkt_d[dt * 128:(dt + 1) * 128, :])
                    for dt in range(DT):
                        nc.sync.dma_start(w_sb["v"][:, dt, :],
                                          wvt_d[dt * 128:(dt + 1) * 128, :])
                for h in range(HPC):
                    ps = {}
                    for t in ("q", "k", "v"):
                        ps[t] = qkvps.tile([128, SC_W], fp32, tag="qkv", name=f"ps_{t}")
                    for t in ("q", "k", "v"):
                        for dt in range(DT):
                            nc.tensor.matmul(
                                ps[t][:],
                                w_sb[t][:, dt, h * HD:(h + 1) * HD],
                                xts[dt][:],
                                start=(dt == 0), stop=(dt == DT - 1))
                    # RoPE for q, k
                    for t, dram, scale in (("q", qT_d, SCALE), ("k", kT_d, 1.0)):
                        til = spool.tile([128, SC_W], bf16, tag="til")
                        nc.scalar.activation(til[:], ps[t][:], Copy,
                                             scale=scale)
                        rp = rotps.tile([128, SC_W], fp32, tag="rot")
                        nc.tensor.matmul(rp[:], rot_sb[:], til[:],
                                         start=True, stop=True)
                        t1 = tpool.tile([128, SC_W], bf16, tag="t1")
                        nc.vector.tensor_mul(t1[:], til[:],
                                             cos_sb[:, s0:s0 + SC_W])
                        hat = spool.tile([128, SC_W], bf16, tag="hat")
                        nc.vector.tensor_mul(hat[:], rp[:],
                                             sin_sb[:, s0:s0 + SC_W])
                        nc.vector.tensor_add(hat[:], hat[:], t1[:])
                        nc.sync.dma_start(dram[bb][h, :, c0:c0 + SC_W], hat[:])
                    # V: copy out of PSUM, then PE-transpose to natural [k, hd]
                    vb = spool.tile([128, SC_W], bf16, tag="vb")
                    nc.scalar.copy(vb[:], ps["v"][:])
                    for vt in range(SC_W // 128):
                        vtp = rotps.tile([128, 128], fp32, tag="rot", name="vtp")
                        nc.tensor.matmul(vtp[:], vb[:, vt * 128:(vt + 1) * 128],
                                         idn_sb[:], start=True, stop=True)
                        vnt = spool.tile([128, 128], bf16, tag="vnt", name="vnt")
                        nc.scalar.copy(vnt[:], vtp[:])
                        nc.sync.dma_start(
                            vN_d[bb][h, c0 + vt * 128: c0 + (vt + 1) * 128, :],
                            vnt[:])

            rotps.release()
            qkvps.release()

            # ================= phase 2: attention + O-proj + RS ===========
            with ExitStack() as _stk2:
                scps = _stk2.enter_context(tc.tile_pool(name="scps", bufs=2, space="PSUM"))
                bps = _stk2.enter_context(tc.tile_pool(name="bps", bufs=1, space="PSUM"))
                sups = _stk2.enter_context(tc.tile_pool(name="sups", bufs=1, space="PSUM"))
                ops = _stk2.enter_context(tc.tile_pool(name="ops", bufs=2, space="PSUM"))
                pps = _stk2.enter_context(tc.tile_pool(name="pps", bufs=2, space="PSUM"))
                def attn_group2(qg, kmax, qT, kT, vbf, oT):
                    pT = {h: ptpool.tile([128, NQT, 512], bf16, tag="pT",
                                         name=f"pT{h}") for h in range(HPC)}
                    po = {h: ops.tile([128, 512], fp32, tag="po",
                                      name=f"po{h}") for h in range(HPC)}
                    for kt in range(kmax + 1):
                        qlo = max(0, kt - qg * 4) * 128
                        n = 512 - qlo
                        for h in range(HPC):
                            sp = scps.tile([128, 512], fp32, tag="sc", name="sp")
                            nc.tensor.matmul(
                                sp[:, :n],
                                kT[h][:, kt * 128:(kt + 1) * 128],
                                qT[h][:, qg * 512 + qlo: (qg + 1) * 512],
                                start=True, stop=True)
                            if kt >= qg * 4:  # diag tile at local cols 0:128
                                nc.vector.tensor_add(
                                    sp[:, 0:128], sp[:, 0:128], mdg_sb[:, kt, :])
                            nc.scalar.activation(
                                pT[h][:, kt, qlo:512], sp[:, :n], Exp)
                            if kt >= 1:
                                klast = kt - 1
                                ql2 = max(0, klast - qg * 4) * 128
                                nc.tensor.matmul(
                                    po[h][:, ql2:512], vbf[h][:, klast, :],
                                    pT[h][:, klast, ql2:512],
                                    start=(klast == 0), stop=False)
                    for h in range(HPC):
                        nc.tensor.matmul(
                            po[h][:, 384:512], vbf[h][:, kmax, :],
                            pT[h][:, kmax, 384:512], start=False, stop=True)
                        sums_ps = sups.tile([1, 512], fp32, tag="sps", name="sums_ps")
                        for kt in range(kmax + 1):
                            qlo = max(0, kt - qg * 4) * 128
                            nc.tensor.matmul(
                                sums_ps[:, qlo:512], one_sb[:],
                                pT[h][:, kt, qlo:512],
                                start=(kt == 0), stop=(kt == kmax))
                        srow = smpool.tile([1, 512], fp32, tag="srow", name="srow")
                        nc.scalar.copy(srow[:], sums_ps[:])
                        sbc_ps = bps.tile([128, 512], fp32, tag="sbc", name="sbc_ps")
                        nc.tensor.matmul(sbc_ps[:], onr_sb[:], srow[:],
                                         start=True, stop=True)
                        rbc = smpool.tile([128, 512], fp32, tag="rbc", name="rbc")
                        nc.vector.reciprocal_approx_fast(rbc[:], sbc_ps[:])
                        nc.vector.tensor_mul(
                            oT[h][:, qg * 512:(qg + 1) * 512], po[h][:], rbc[:])

                qTa, kTa, vbfa = {}, {}, {}
                for bb2 in range(B):
                    for h in range(HPC):
                        qTa[(bb2, h)] = qkpool.tile([128, S], bf16, tag=f"qT{h}", name=f"qT{bb2}{h}")
                        nc.sync.dma_start(qTa[(bb2, h)][:], qT_d[bb2][h])
                        kTa[(bb2, h)] = qkpool.tile([128, S], bf16, tag=f"kT{h}", name=f"kT{bb2}{h}")
                        nc.sync.dma_start(kTa[(bb2, h)][:], kT_d[bb2][h])
                        vbfa[(bb2, h)] = vpool.tile([128, NQT, HD], bf16, tag=f"v{h}", name=f"v{bb2}{h}")
                        nc.sync.dma_start(
                            vbfa[(bb2, h)][:],
                            vN_d[bb2][h].rearrange("(t p) d -> p t d", p=128))

                oTa = {}
                for bb2 in range(B):
                    for h in range(HPC):
                        oTa[(bb2, h)] = opool.tile([128, S], bf16, tag=f"oT{h}",
                                                   name=f"oT{bb2}{h}")

                for qg in range(NQG):
                    kmax = qg * 4 + 3
                    for b in range(B):
                        qT = {h: qTa[(b, h)] for h in range(HPC)}
                        kT = {h: kTa[(b, h)] for h in range(HPC)}
                        vbf = {h: vbfa[(b, h)] for h in range(HPC)}
                        oT = {h: oTa[(b, h)] for h in range(HPC)}
                        attn_group2(qg, kmax, qT, kT, vbf, oT)

                        # ---- O-projection for this q-group + ReduceScatter ----
                        for st in range(qg * 4, qg * 4 + 4):
                            pp = [pps.tile([128, 512], fp32, tag="pp", name=f"pp{e}") for e in range(4)]
                            for h in range(HPC):
                                for ec in range(4):
                                    nc.tensor.matmul(
                                        pp[ec][:],
                                        oT[h][:, st * 128:(st + 1) * 128],
                                        wot_sb[:, h, ec * 512:(ec + 1) * 512],
                                        start=(h == 0), stop=(h == HPC - 1))
                            par = papool.tile([128, DIM], bf16, tag="par")
                            chx, r0 = divmod(b * S + st * 128, RS_ROWS)
                            for ec in range(4):
                                if ec % 2 == 0:
                                    nc.scalar.copy(par[:, ec * 512:(ec + 1) * 512], pp[ec][:])
                                else:
                                    nc.vector.tensor_copy(par[:, ec * 512:(ec + 1) * 512], pp[ec][:])
                            nc.sync.dma_start(par_d[chx][r0:r0 + 128, :], par[:])
                            if r0 + 128 == RS_ROWS:
                                nc.gpsimd.collective_compute(
                                    "ReduceScatter", ADD, replica_groups=rg,
                                    ins=[par_d[chx][:]],
                                    outs=[rs_d[chx][:]])
                                nc.gpsimd.dma_start(out_d[chx], rs_d[chx][:])

    nc.compile()
    return nc


def _get_nc(S):
    if S not in _CACHE:
        _CACHE[S] = _build(S)
    return _CACHE[S]


def make_inputs(x, freqs_cis, mask, wq, wk, wv, wo):
    """Host-side sharding / layout prep. Returns in_maps for 8 cores."""
    S = x.shape[1]
    flat_xt = np.ascontiguousarray(np.asarray(x, np.float32).reshape(B * S, DIM).T)
    cos = np.asarray(freqs_cis[..., 0], np.float32)   # [S, HD/2]
    sin = np.asarray(freqs_cis[..., 1], np.float32)
    cos_t = np.ascontiguousarray(np.repeat(cos.T, 2, axis=0))  # [HD, S]
    sin_t = np.ascontiguousarray(np.repeat(sin.T, 2, axis=0))
    m = np.asarray(mask, np.float32)[0, 0]
    nqt = S // 128
    mask_diag = np.ascontiguousarray(
        np.stack([m[i * 128:(i + 1) * 128, i * 128:(i + 1) * 128].T
                  for i in range(nqt)]))
    import ml_dtypes
    bf = ml_dtypes.bfloat16
    flat_xt = flat_xt.astype(bf)
    cos_t = cos_t.astype(bf)
    sin_t = sin_t.astype(bf)
    ident_bf = np.eye(128, dtype=bf)
    P = np.zeros((128, 128), np.float32)
    for j in range(64):
        P[2 * j, 2 * j + 1] = -1.0
        P[2 * j + 1, 2 * j] = 1.0
    rotp = np.ascontiguousarray(P.T)

    in_maps = []
    for c in range(N_CORES):
        r = slice(c * OC, (c + 1) * OC)
        in_maps.append({
            "xt": flat_xt,
            "wqt": np.ascontiguousarray(np.asarray(wq, np.float32)[r, :].T).astype(bf),
            "wkt": np.ascontiguousarray(np.asarray(wk, np.float32)[r, :].T).astype(bf),
            "wvt": np.ascontiguousarray(np.asarray(wv, np.float32)[r, :].T).astype(bf),
            "wot": np.ascontiguousarray(np.asarray(wo, np.float32)[:, r].T).astype(bf),
            "cos_t": cos_t,
            "sin_t": sin_t,
            "mask_diag": mask_diag,
            "ident_bf": ident_bf,
            "rotp": rotp.astype(bf),
            "ones_bf": np.ones((128, 1), dtype=bf),
            "ones_row": np.ones((1, 128), dtype=np.float32),
        })
    return in_maps


def assemble(results, S):
    """Concatenate per-core ReduceScatter shards into the full output."""
    nch = B * S // RS_ROWS
    per = RS_ROWS // N_CORES
    full = np.empty((nch, N_CORES, per, DIM), np.float32)
    for c in range(N_CORES):
        full[:, c] = np.asarray(results[c]["out"], np.float32).reshape(nch, per, DIM)
    return full.reshape(B, S, DIM)


def kernel(x, start_pos, freqs_cis, mask, wq, wk, wv, wo):
    from concourse.bass_utils import run_bass_kernel_spmd
    S = x.shape[1]
    nc = _get_nc(S)
    in_maps = make_inputs(x, freqs_cis, mask, wq, wk, wv, wo)
    res = run_bass_kernel_spmd(nc, in_maps, core_ids=list(range(N_CORES)))
    return assemble(res.results, S)
